# revision 4
# baseline (speedup 1.0000x reference)
"""Trainium2 Bass kernel for nn_CrossAttention2D (v3: host-T2, fp16, wide exp).

Reference computation (per batch b, row h):
    Q = w1 @ Xw + b1          (Xw = waveform[b,:,h,:]  [C=128, W=512])
    K = w2 @ Xs + b2          (Xs = spectrogram[b,:,h,:])
    S = Q^T K * 1/sqrt(F)     [512, 512]
    P = softmax(S, axis=-1)
    out[b,:,h,:] = Xs @ P^T   [C, W]

Device algorithm (S^T layout so softmax needs no transposes):
    S^T = Xs^T T2'   with  T2' = (w1^T w2)^T Xw + (w2^T b1) 1^T
precomputed on the HOST (it is a tiny [128,128] map over the waveform and
folds the only softmax-surviving bias), shipped as fp16.  Per row h:
    st[p] = Xs[:,pair p]^T T2'        (4 matmuls into 2x [128,2,512] PSUM)
    P~[p] = exp(scale*st[p] - 3)      (2 wide ACT exps, fp16 out; the -3
                                       cancels in the softmax ratio)
    rb   += ones^T P~[p]              (4 matmuls, row sums broadcast to 128)
    o    += V^T[kc] P~[p]             (4 matmuls, V^T host-pretransposed fp16)
    out   = o / rb                    (single DVE divide, fp16 out)
All device matmuls are fp16 (10-bit mantissa beats bf16 at identical PE
cost); accumulation is fp32 in PSUM.  Output is fp16, host-upcast.

The emission is software-pipelined one h ahead: body(i) issues divide(i-1)
first on the otherwise-idle DVE, then the score matmuls + exps of row i+1,
then the AV/row-sum matmuls of row i.  The PE stream is 12 N=512 matmuls
per row with no PSUM->SBUF copies on the critical path.

Sharding: data-parallel over batch B=8 across 8 NeuronCores (one batch
image per core, small weights replicated). No collectives.
"""

import contextlib

import numpy as np

import concourse.bacc as bacc
import concourse.tile as tile
from concourse import mybir
from concourse.bass_utils import run_bass_kernel_spmd

B = 8
C = 128  # channel dim (TIME_DIM == SPEC_DIM == 128)
H = 64
W = 512
N_CORES = 8
SCALE = 1.0 / 16.0  # 1/sqrt(FEATURE_DIM=256)
SHIFT = -3.0  # exp range guard; cancels in softmax
HB = 8  # h rows per input DMA block
OB = 4  # h rows per output DMA block

FP32 = mybir.dt.float32
FP16 = mybir.dt.float16
EXP = mybir.ActivationFunctionType.Exp
DIV = mybir.AluOpType.divide
MULT = mybir.AluOpType.mult
USE_DIV = False  # walrus BIR verifier rejects AluOpType.divide on DVE


def build_module(n_h=H, rep=1):
    """Build the per-core Bass module processing [C, n_h, W] inputs."""
    assert n_h % HB == 0 and HB % OB == 0 and n_h >= 2 * HB
    nc = bacc.Bacc("TRN2", target_bir_lowering=False, debug=False)

    t2p = nc.dram_tensor("t2p", [C, n_h, W], FP16, kind="ExternalInput").ap()
    spec = nc.dram_tensor("spec", [C, n_h, W], FP16, kind="ExternalInput").ap()
    # V^T blocks: st4[w0, h, j, c] = spec[c, h, 128*j + w0]
    st4 = nc.dram_tensor("st4", [C, n_h, 4, C], FP16, kind="ExternalInput").ap()
    out = nc.dram_tensor("out", [C, n_h, W], FP16, kind="ExternalOutput").ap()

    n_blk = n_h // HB

    with tile.TileContext(nc) as tc:
        with (
            tc.tile_pool(name="consts", bufs=1) as consts,
            tc.tile_pool(name="io", bufs=4) as io,
            tc.tile_pool(name="ob", bufs=2) as ob,
            tc.tile_pool(name="work", bufs=4) as work,
            tc.tile_pool(name="ps", bufs=1, space="PSUM") as ps,
        ):
            ones_sb = consts.tile([C, C], FP16, tag="ones")
            nc.vector.memset(ones_sb, 1.0)
            shift_sb = consts.tile([C, 1], FP32, tag="shift")
            nc.vector.memset(shift_sb, SHIFT)

            blocks = {}  # blk index -> (t2p, spec, st4) tiles

            def load_blk(blk):
                h0 = blk * HB
                t2_t = io.tile([C, HB, W], FP16, tag="t2", name="t2_t")
                nc.sync.dma_start(t2_t, t2p[:, h0 : h0 + HB, :])
                sp_t = io.tile([C, HB, W], FP16, tag="sp", name="sp_t")
                nc.sync.dma_start(sp_t, spec[:, h0 : h0 + HB, :])
                s4_t = io.tile([C, HB, 4, C], FP16, tag="s4", name="s4_t")
                nc.sync.dma_start(s4_t, st4[:, h0 : h0 + HB, :, :])
                blocks[blk] = (t2_t, sp_t, s4_t)

            def st_pair(h, p, expst):
                """Two score matmuls + one wide exp for chunk pair p of row h."""
                t2 = blocks[h // HB][0][:, h % HB, :]
                sp = blocks[h // HB][1][:, h % HB, :]
                st_ps = ps.tile([C, 2, W], FP32, tag="st", bufs=2)
                for j in range(2):
                    kc = 2 * p + j
                    kblk = slice(kc * 128, (kc + 1) * 128)
                    nc.tensor.matmul(
                        st_ps[:, j, :], sp[:, kblk], t2, start=True, stop=True
                    )
                nc.scalar.activation(
                    expst[:, p, :, :], st_ps, EXP, bias=shift_sb, scale=SCALE
                )

            # ---- prologue: fill the pipeline ----
            load_blk(0)
            load_blk(1)
            cur_exp = work.tile([C, 2, 2, W], FP16, tag="p", name="expst")
            for p in range(2):
                st_pair(0, p, cur_exp)

            o_blks = {}
            finish_prev = None  # closure finalizing row i-1 (divide + dma)

            rep_ctx = tc.For_i(0, rep, 1) if rep > 1 else contextlib.nullcontext()
            with rep_ctx:
                for i in range(n_h):
                    if i % HB == 0:
                        load_blk((i // HB + 2) % n_blk)
                    if i % OB == 0:
                        o_blks[i // OB] = ob.tile(
                            [C, OB, W], FP16, tag="o", name="o_blk"
                        )
                        if i // OB - 2 in o_blks:
                            del o_blks[i // OB - 2]
                    # row i-1 epilogue first: the idle DVE runs it while the
                    # PE streams row i+1's score matmuls
                    if finish_prev is not None:
                        finish_prev()
                        finish_prev = None

                    expst_i = cur_exp
                    nxt_exp = work.tile([C, 2, 2, W], FP16, tag="p", name="expst")
                    for p in range(2):
                        st_pair((i + 1) % n_h, p, nxt_exp)

                    st4_i = blocks[i // HB][2]
                    o_ps = ps.tile([C, W], FP32, tag="o", bufs=2)
                    for kc in range(4):
                        nc.tensor.matmul(
                            o_ps,
                            st4_i[:, i % HB, kc, :],
                            expst_i[:, kc // 2, kc % 2, :],
                            start=(kc == 0),
                            stop=(kc == 3),
                        )
                    rb_ps = ps.tile([C, W], FP32, tag="rb", bufs=2)
                    for kc in range(4):
                        nc.tensor.matmul(
                            rb_ps,
                            ones_sb,
                            expst_i[:, kc // 2, kc % 2, :],
                            start=(kc == 0),
                            stop=(kc == 3),
                        )
                    cur_exp = nxt_exp

                    def finish_row(i=i, rb_ps=rb_ps, o_ps=o_ps):
                        o_blk = o_blks[i // OB]
                        if USE_DIV:
                            nc.vector.tensor_tensor(
                                o_blk[:, i % OB, :], o_ps, rb_ps, op=DIV
                            )
                        else:
                            rcb_sb = work.tile([C, W], FP32, tag="rcb")
                            nc.vector.reciprocal(rcb_sb, rb_ps)
                            nc.vector.tensor_tensor(
                                o_blk[:, i % OB, :], o_ps, rcb_sb, op=MULT
                            )
                        if i % OB == OB - 1:
                            h1 = i - (OB - 1)
                            nc.gpsimd.dma_start(out[:, h1 : h1 + OB, :], o_blk)

                    if i == n_h - 1:
                        # last body finishes itself so the loop body is
                        # self-contained across the For_i back edge
                        finish_row()
                    else:
                        finish_prev = finish_row

    nc.compile()
    return nc


def host_prep(waveform, spectrogram, w1, b1, w2, b2):
    """Precompute host-side tensors.

    T2' = (w1^T w2)^T Xw + (w2^T b1) 1^T  folds the 1x1-conv weights and the
    softmax-surviving bias into a single fp16 tensor shipped to the device.
    """
    w1d = np.asarray(w1, np.float64)
    w2d = np.asarray(w2, np.float64)
    b1d = np.asarray(b1, np.float64)
    mt = (w1d.T @ w2d).astype(np.float32)  # [c1, c2]
    beta = (w2d.T @ b1d).astype(np.float32)  # [c2]
    wave = np.asarray(waveform, np.float32).reshape(B, C, H * W)
    t2p = (
        (np.einsum("cd,bcx->bdx", mt, wave) + beta[None, :, None])
        .reshape(B, C, H, W)
        .astype(np.float16)
    )
    spec_f = np.asarray(spectrogram, np.float32)
    spec16 = spec_f.astype(np.float16)
    # st4[b, w0, h, j, c] = spec[b, c, h, 128*j + w0]
    st4 = np.ascontiguousarray(
        spec16.reshape(B, C, H, 4, 128).transpose(0, 4, 2, 3, 1)
    )
    return t2p, spec16, st4


_NC_CACHE = {}


def _get_nc(n_h=H, rep=1):
    key = (n_h, rep)
    if key not in _NC_CACHE:
        _NC_CACHE[key] = build_module(n_h, rep)
    return _NC_CACHE[key]


def run_device(waveform, spectrogram, w1, b1, w2, b2, n_h=H, rep=1, **run_kwargs):
    """Shard over batch, run on 8 cores, gather. Returns (output, results)."""
    t2p, spec16, st4 = host_prep(waveform, spectrogram, w1, b1, w2, b2)

    in_maps = [
        {
            "t2p": np.ascontiguousarray(t2p[b, :, :n_h, :]),
            "spec": np.ascontiguousarray(spec16[b, :, :n_h, :]),
            "st4": np.ascontiguousarray(st4[b, :, :n_h, :, :]),
        }
        for b in range(B)
    ]
    nc = _get_nc(n_h, rep)
    res = run_bass_kernel_spmd(nc, in_maps, core_ids=list(range(N_CORES)), **run_kwargs)
    output = np.stack([res.results[b]["out"] for b in range(B)], axis=0)
    return output, res


def kernel(waveform, spectrogram, w1, b1, w2, b2):
    output, _ = run_device(waveform, spectrogram, w1, b1, w2, b2)
    return output.astype(np.float32)


# revision 6
# speedup vs baseline: 1.1286x; 1.1286x over previous
"""Trainium2 Bass kernel for nn_CrossAttention2D (v3: host-T2, fp16, wide exp).

Reference computation (per batch b, row h):
    Q = w1 @ Xw + b1          (Xw = waveform[b,:,h,:]  [C=128, W=512])
    K = w2 @ Xs + b2          (Xs = spectrogram[b,:,h,:])
    S = Q^T K * 1/sqrt(F)     [512, 512]
    P = softmax(S, axis=-1)
    out[b,:,h,:] = Xs @ P^T   [C, W]

Device algorithm (S^T layout so softmax needs no transposes):
    S^T = Xs^T T2'   with  T2' = (w1^T w2)^T Xw + (w2^T b1) 1^T
precomputed on the HOST (it is a tiny [128,128] map over the waveform and
folds the only softmax-surviving bias), shipped as fp16.  Per row h:
    st[p] = Xs[:,pair p]^T T2'        (4 matmuls into 2x [128,2,512] PSUM)
    P~[p] = exp(scale*st[p] - 3)      (2 wide ACT exps, fp16 out; the -3
                                       cancels in the softmax ratio)
    rb   += ones^T P~[p]              (4 matmuls, row sums broadcast to 128)
    o    += V^T[kc] P~[p]             (4 matmuls, V^T host-pretransposed fp16)
    out   = o / rb                    (single DVE divide, fp16 out)
All device matmuls are fp16 (10-bit mantissa beats bf16 at identical PE
cost); accumulation is fp32 in PSUM.  Output is fp16, host-upcast.

The emission is software-pipelined one h ahead: body(i) issues divide(i-1)
first on the otherwise-idle DVE, then the score matmuls + exps of row i+1,
then the AV/row-sum matmuls of row i.  The PE stream is 12 N=512 matmuls
per row with no PSUM->SBUF copies on the critical path.

Sharding: data-parallel over batch B=8 across 8 NeuronCores (one batch
image per core, small weights replicated). No collectives.
"""

import contextlib

import numpy as np

import concourse.bacc as bacc
import concourse.tile as tile
from concourse import mybir
from concourse.bass_utils import run_bass_kernel_spmd

B = 8
C = 128  # channel dim (TIME_DIM == SPEC_DIM == 128)
H = 64
W = 512
N_CORES = 8
SCALE = 1.0 / 16.0  # 1/sqrt(FEATURE_DIM=256)
SHIFT = -3.0  # exp range guard; cancels in softmax
HB = 8  # h rows per input DMA block
OB = 4  # h rows per output DMA block

FP32 = mybir.dt.float32
FP16 = mybir.dt.float16
EXP = mybir.ActivationFunctionType.Exp
DIV = mybir.AluOpType.divide
MULT = mybir.AluOpType.mult
USE_DIV = False  # walrus BIR verifier rejects AluOpType.divide on DVE


def build_module(n_h=H, rep=1, dt=FP16, wide_exp=True):
    """Build the per-core Bass module processing [C, n_h, W] inputs."""
    assert n_h % HB == 0 and HB % OB == 0 and n_h >= 2 * HB
    nc = bacc.Bacc("TRN2", target_bir_lowering=False, debug=False)

    t2p = nc.dram_tensor("t2p", [C, n_h, W], dt, kind="ExternalInput").ap()
    spec = nc.dram_tensor("spec", [C, n_h, W], dt, kind="ExternalInput").ap()
    # V^T blocks: st4[w0, h, j, c] = spec[c, h, 128*j + w0]
    st4 = nc.dram_tensor("st4", [C, n_h, 4, C], dt, kind="ExternalInput").ap()
    out = nc.dram_tensor("out", [C, n_h, W], FP16, kind="ExternalOutput").ap()

    n_blk = n_h // HB

    with tile.TileContext(nc) as tc:
        with (
            tc.tile_pool(name="consts", bufs=1) as consts,
            tc.tile_pool(name="io", bufs=4) as io,
            tc.tile_pool(name="ob", bufs=2) as ob,
            tc.tile_pool(name="work", bufs=4) as work,
            tc.tile_pool(name="ps", bufs=1, space="PSUM") as ps,
        ):
            ones_sb = consts.tile([C, C], dt, tag="ones")
            nc.vector.memset(ones_sb, 1.0)
            shift_sb = consts.tile([C, 1], FP32, tag="shift")
            nc.vector.memset(shift_sb, SHIFT)

            blocks = {}  # blk index -> (t2p, spec, st4) tiles

            def load_blk(blk):
                h0 = blk * HB
                t2_t = io.tile([C, HB, W], dt, tag="t2", name="t2_t")
                nc.sync.dma_start(t2_t, t2p[:, h0 : h0 + HB, :])
                sp_t = io.tile([C, HB, W], dt, tag="sp", name="sp_t")
                nc.sync.dma_start(sp_t, spec[:, h0 : h0 + HB, :])
                s4_t = io.tile([C, HB, 4, C], dt, tag="s4", name="s4_t")
                nc.sync.dma_start(s4_t, st4[:, h0 : h0 + HB, :, :])
                blocks[blk] = (t2_t, sp_t, s4_t)

            def st_pair(h, p, expst):
                """Two score matmuls + exp(s) for chunk pair p of row h."""
                t2 = blocks[h // HB][0][:, h % HB, :]
                sp = blocks[h // HB][1][:, h % HB, :]
                nsub = 2 if wide_exp else 1
                for s in range(2 // nsub):
                    st_ps = ps.tile([C, nsub, W], FP32, tag="st", bufs=2 * (2 // nsub))
                    for j in range(nsub):
                        kc = 2 * p + s * nsub + j
                        kblk = slice(kc * 128, (kc + 1) * 128)
                        nc.tensor.matmul(
                            st_ps[:, j, :], sp[:, kblk], t2, start=True, stop=True
                        )
                    nc.scalar.activation(
                        expst[:, p, s * nsub : (s + 1) * nsub, :].reshape(
                            [C, nsub, W]
                        ),
                        st_ps,
                        EXP,
                        bias=shift_sb,
                        scale=SCALE,
                    )

            # ---- prologue: fill the pipeline ----
            load_blk(0)
            load_blk(1)
            cur_exp = work.tile([C, 2, 2, W], dt, tag="p", name="expst")
            for p in range(2):
                st_pair(0, p, cur_exp)

            o_blks = {}
            finish_prev = None  # closure finalizing row i-1 (divide + dma)

            rep_ctx = tc.For_i(0, rep, 1) if rep > 1 else contextlib.nullcontext()
            with rep_ctx:
                for i in range(n_h):
                    if i % HB == 0:
                        load_blk((i // HB + 2) % n_blk)
                    if i % OB == 0:
                        o_blks[i // OB] = ob.tile(
                            [C, OB, W], FP16, tag="o", name="o_blk"
                        )
                        if i // OB - 2 in o_blks:
                            del o_blks[i // OB - 2]
                    # row i-1 epilogue first: the idle DVE runs it while the
                    # PE streams row i+1's score matmuls
                    if finish_prev is not None:
                        finish_prev()
                        finish_prev = None

                    expst_i = cur_exp
                    nxt_exp = work.tile([C, 2, 2, W], dt, tag="p", name="expst")
                    for p in range(2):
                        st_pair((i + 1) % n_h, p, nxt_exp)

                    st4_i = blocks[i // HB][2]
                    o_ps = ps.tile([C, W], FP32, tag="o", bufs=2)
                    for kc in range(4):
                        nc.tensor.matmul(
                            o_ps,
                            st4_i[:, i % HB, kc, :],
                            expst_i[:, kc // 2, kc % 2, :],
                            start=(kc == 0),
                            stop=(kc == 3),
                        )
                    rb_ps = ps.tile([C, W], FP32, tag="rb", bufs=2)
                    for kc in range(4):
                        nc.tensor.matmul(
                            rb_ps,
                            ones_sb,
                            expst_i[:, kc // 2, kc % 2, :],
                            start=(kc == 0),
                            stop=(kc == 3),
                        )
                    cur_exp = nxt_exp

                    def finish_row(i=i, rb_ps=rb_ps, o_ps=o_ps):
                        o_blk = o_blks[i // OB]
                        if USE_DIV:
                            nc.vector.tensor_tensor(
                                o_blk[:, i % OB, :], o_ps, rb_ps, op=DIV
                            )
                        else:
                            rcb_sb = work.tile([C, W], FP32, tag="rcb")
                            nc.vector.reciprocal(rcb_sb, rb_ps)
                            nc.vector.tensor_tensor(
                                o_blk[:, i % OB, :], o_ps, rcb_sb, op=MULT
                            )
                        if i % OB == OB - 1:
                            h1 = i - (OB - 1)
                            nc.gpsimd.dma_start(out[:, h1 : h1 + OB, :], o_blk)

                    if i == n_h - 1:
                        # last body finishes itself so the loop body is
                        # self-contained across the For_i back edge
                        finish_row()
                    else:
                        finish_prev = finish_row

    nc.compile()
    return nc


def host_prep(waveform, spectrogram, w1, b1, w2, b2, np_dt=np.float16):
    """Precompute host-side tensors.

    T2' = (w1^T w2)^T Xw + (w2^T b1) 1^T  folds the 1x1-conv weights and the
    softmax-surviving bias into a single fp16 tensor shipped to the device.
    """
    w1d = np.asarray(w1, np.float64)
    w2d = np.asarray(w2, np.float64)
    b1d = np.asarray(b1, np.float64)
    mt = (w1d.T @ w2d).astype(np.float32)  # [c1, c2]
    beta = (w2d.T @ b1d).astype(np.float32)  # [c2]
    wave = np.asarray(waveform, np.float32).reshape(B, C, H * W)
    import ml_dtypes

    if np_dt is None:
        np_dt = ml_dtypes.bfloat16
    t2p = (
        (np.einsum("cd,bcx->bdx", mt, wave) + beta[None, :, None])
        .reshape(B, C, H, W)
        .astype(np_dt)
    )
    spec_f = np.asarray(spectrogram, np.float32)
    spec16 = spec_f.astype(np_dt)
    # st4[b, w0, h, j, c] = spec[b, c, h, 128*j + w0]
    st4 = np.ascontiguousarray(
        spec16.reshape(B, C, H, 4, 128).transpose(0, 4, 2, 3, 1)
    )
    return t2p, spec16, st4


_NC_CACHE = {}


def _get_nc(n_h=H, rep=1, **bkw):
    key = (n_h, rep, tuple(sorted(bkw.items())))
    if key not in _NC_CACHE:
        _NC_CACHE[key] = build_module(n_h, rep, **bkw)
    return _NC_CACHE[key]


def run_device(
    waveform, spectrogram, w1, b1, w2, b2, n_h=H, rep=1, bkw=None, **run_kwargs
):
    """Shard over batch, run on 8 cores, gather. Returns (output, results)."""
    bkw = bkw or {}
    np_dt = np.float16 if bkw.get("dt", FP16) == FP16 else None
    t2p, spec16, st4 = host_prep(waveform, spectrogram, w1, b1, w2, b2, np_dt)

    in_maps = [
        {
            "t2p": np.ascontiguousarray(t2p[b, :, :n_h, :]),
            "spec": np.ascontiguousarray(spec16[b, :, :n_h, :]),
            "st4": np.ascontiguousarray(st4[b, :, :n_h, :, :]),
        }
        for b in range(B)
    ]
    nc = _get_nc(n_h, rep, **bkw)
    res = run_bass_kernel_spmd(nc, in_maps, core_ids=list(range(N_CORES)), **run_kwargs)
    output = np.stack([res.results[b]["out"] for b in range(B)], axis=0)
    return output, res


def kernel(waveform, spectrogram, w1, b1, w2, b2):
    output, _ = run_device(waveform, spectrogram, w1, b1, w2, b2)
    return output.astype(np.float32)


# revision 7
# speedup vs baseline: 1.6588x; 1.4698x over previous
"""Trainium2 Bass kernel for nn_CrossAttention2D (v3: host-T2, fp16, wide exp).

Reference computation (per batch b, row h):
    Q = w1 @ Xw + b1          (Xw = waveform[b,:,h,:]  [C=128, W=512])
    K = w2 @ Xs + b2          (Xs = spectrogram[b,:,h,:])
    S = Q^T K * 1/sqrt(F)     [512, 512]
    P = softmax(S, axis=-1)
    out[b,:,h,:] = Xs @ P^T   [C, W]

Device algorithm (S^T layout so softmax needs no transposes):
    S^T = Xs^T T2'   with  T2' = (w1^T w2)^T Xw + (w2^T b1) 1^T
precomputed on the HOST (it is a tiny [128,128] map over the waveform and
folds the only softmax-surviving bias), shipped as fp16.  Per row h:
    st[p] = Xs[:,pair p]^T T2'        (4 matmuls into 2x [128,2,512] PSUM)
    P~[p] = exp(scale*st[p] - 3)      (2 wide ACT exps, fp16 out; the -3
                                       cancels in the softmax ratio)
    rb   += ones^T P~[p]              (4 matmuls, row sums broadcast to 128)
    o    += V^T[kc] P~[p]             (4 matmuls, V^T host-pretransposed fp16)
    out   = o / rb                    (single DVE divide, fp16 out)
All device matmuls are fp16 (10-bit mantissa beats bf16 at identical PE
cost); accumulation is fp32 in PSUM.  Output is fp16, host-upcast.

The emission is software-pipelined one h ahead: body(i) issues divide(i-1)
first on the otherwise-idle DVE, then the score matmuls + exps of row i+1,
then the AV/row-sum matmuls of row i.  The PE stream is 12 N=512 matmuls
per row with no PSUM->SBUF copies on the critical path.

Sharding: data-parallel over batch B=8 across 8 NeuronCores (one batch
image per core, small weights replicated). No collectives.
"""

import contextlib

import numpy as np

import concourse.bacc as bacc
import concourse.tile as tile
from concourse import mybir
from concourse.bass_utils import run_bass_kernel_spmd

B = 8
C = 128  # channel dim (TIME_DIM == SPEC_DIM == 128)
H = 64
W = 512
N_CORES = 8
SCALE = 1.0 / 16.0  # 1/sqrt(FEATURE_DIM=256)
SHIFT = -3.0  # exp range guard; cancels in softmax
HB = 8  # h rows per input DMA block
OB = 4  # h rows per output DMA block

FP32 = mybir.dt.float32
FP16 = mybir.dt.float16
EXP = mybir.ActivationFunctionType.Exp
DIV = mybir.AluOpType.divide
MULT = mybir.AluOpType.mult
USE_DIV = False  # walrus BIR verifier rejects AluOpType.divide on DVE


def build_module(n_h=H, rep=1, dt=FP16, wide_exp=True):
    """Build the per-core Bass module processing [C, n_h, W] inputs."""
    assert n_h % HB == 0 and HB % OB == 0 and n_h >= 2 * HB
    nc = bacc.Bacc("TRN2", target_bir_lowering=False, debug=False)

    t2p = nc.dram_tensor("t2p", [C, n_h, W], dt, kind="ExternalInput").ap()
    spec = nc.dram_tensor("spec", [C, n_h, W], dt, kind="ExternalInput").ap()
    # V^T blocks: st4[w0, h, j, c] = spec[c, h, 128*j + w0]
    st4 = nc.dram_tensor("st4", [C, n_h, 4, C], dt, kind="ExternalInput").ap()
    out = nc.dram_tensor("out", [C, n_h, W], FP16, kind="ExternalOutput").ap()

    n_blk = n_h // HB

    with tile.TileContext(nc) as tc:
        with (
            tc.tile_pool(name="consts", bufs=1) as consts,
            tc.tile_pool(name="io", bufs=4) as io,
            tc.tile_pool(name="ob", bufs=2) as ob,
            tc.tile_pool(name="work", bufs=4) as work,
            tc.tile_pool(name="ps", bufs=1, space="PSUM") as ps,
        ):
            ones_sb = consts.tile([C, C], dt, tag="ones")
            nc.vector.memset(ones_sb, 1.0)
            shift_sb = consts.tile([C, 1], FP32, tag="shift")
            nc.vector.memset(shift_sb, SHIFT)

            blocks = {}  # blk index -> (t2p, spec, st4) tiles

            def load_blk(blk):
                h0 = blk * HB
                t2_t = io.tile([C, HB, W], dt, tag="t2", name="t2_t")
                nc.sync.dma_start(t2_t, t2p[:, h0 : h0 + HB, :])
                sp_t = io.tile([C, HB, W], dt, tag="sp", name="sp_t")
                nc.sync.dma_start(sp_t, spec[:, h0 : h0 + HB, :])
                s4_t = io.tile([C, HB, 4, C], dt, tag="s4", name="s4_t")
                nc.sync.dma_start(s4_t, st4[:, h0 : h0 + HB, :, :])
                blocks[blk] = (t2_t, sp_t, s4_t)

            def st_pair(h, p, expst):
                """Two score matmuls + exp(s) for chunk pair p of row h."""
                t2 = blocks[h // HB][0][:, h % HB, :]
                sp = blocks[h // HB][1][:, h % HB, :]
                nsub = 2 if wide_exp else 1
                for s in range(2 // nsub):
                    st_ps = ps.tile([C, nsub, W], FP32, tag="st", bufs=2 * (2 // nsub))
                    for j in range(nsub):
                        kc = 2 * p + s * nsub + j
                        kblk = slice(kc * 128, (kc + 1) * 128)
                        nc.tensor.matmul(
                            st_ps[:, j, :], sp[:, kblk], t2, start=True, stop=True
                        )
                    e_out = expst[:, p, :, :] if nsub == 2 else expst[:, p, s, :]
                    nc.scalar.activation(
                        e_out,
                        st_ps if nsub == 2 else st_ps[:, 0, :],
                        EXP,
                        bias=shift_sb,
                        scale=SCALE,
                    )

            # ---- prologue: fill the pipeline ----
            load_blk(0)
            load_blk(1)
            cur_exp = work.tile([C, 2, 2, W], dt, tag="p", name="expst")
            for p in range(2):
                st_pair(0, p, cur_exp)

            o_blks = {}
            finish_prev = None  # closure finalizing row i-1 (divide + dma)

            rep_ctx = tc.For_i(0, rep, 1) if rep > 1 else contextlib.nullcontext()
            with rep_ctx:
                for i in range(n_h):
                    if i % HB == 0:
                        load_blk((i // HB + 2) % n_blk)
                    if i % OB == 0:
                        o_blks[i // OB] = ob.tile(
                            [C, OB, W], FP16, tag="o", name="o_blk"
                        )
                        if i // OB - 2 in o_blks:
                            del o_blks[i // OB - 2]
                    # row i-1 epilogue first: the idle DVE runs it while the
                    # PE streams row i+1's score matmuls
                    if finish_prev is not None:
                        finish_prev()
                        finish_prev = None

                    expst_i = cur_exp
                    nxt_exp = work.tile([C, 2, 2, W], dt, tag="p", name="expst")
                    for p in range(2):
                        st_pair((i + 1) % n_h, p, nxt_exp)

                    st4_i = blocks[i // HB][2]
                    o_ps = ps.tile([C, W], FP32, tag="o", bufs=2)
                    for kc in range(4):
                        nc.tensor.matmul(
                            o_ps,
                            st4_i[:, i % HB, kc, :],
                            expst_i[:, kc // 2, kc % 2, :],
                            start=(kc == 0),
                            stop=(kc == 3),
                        )
                    rb_ps = ps.tile([C, W], FP32, tag="rb", bufs=2)
                    for kc in range(4):
                        nc.tensor.matmul(
                            rb_ps,
                            ones_sb,
                            expst_i[:, kc // 2, kc % 2, :],
                            start=(kc == 0),
                            stop=(kc == 3),
                        )
                    cur_exp = nxt_exp

                    def finish_row(i=i, rb_ps=rb_ps, o_ps=o_ps):
                        o_blk = o_blks[i // OB]
                        if USE_DIV:
                            nc.vector.tensor_tensor(
                                o_blk[:, i % OB, :], o_ps, rb_ps, op=DIV
                            )
                        else:
                            rcb_sb = work.tile([C, W], FP32, tag="rcb")
                            nc.vector.reciprocal(rcb_sb, rb_ps)
                            nc.vector.tensor_tensor(
                                o_blk[:, i % OB, :], o_ps, rcb_sb, op=MULT
                            )
                        if i % OB == OB - 1:
                            h1 = i - (OB - 1)
                            nc.gpsimd.dma_start(out[:, h1 : h1 + OB, :], o_blk)

                    if i == n_h - 1:
                        # last body finishes itself so the loop body is
                        # self-contained across the For_i back edge
                        finish_row()
                    else:
                        finish_prev = finish_row

    nc.compile()
    return nc


def host_prep(waveform, spectrogram, w1, b1, w2, b2, np_dt=np.float16):
    """Precompute host-side tensors.

    T2' = (w1^T w2)^T Xw + (w2^T b1) 1^T  folds the 1x1-conv weights and the
    softmax-surviving bias into a single fp16 tensor shipped to the device.
    """
    w1d = np.asarray(w1, np.float64)
    w2d = np.asarray(w2, np.float64)
    b1d = np.asarray(b1, np.float64)
    mt = (w1d.T @ w2d).astype(np.float32)  # [c1, c2]
    beta = (w2d.T @ b1d).astype(np.float32)  # [c2]
    wave = np.asarray(waveform, np.float32).reshape(B, C, H * W)
    import ml_dtypes

    if np_dt is None:
        np_dt = ml_dtypes.bfloat16
    t2p = (
        (np.einsum("cd,bcx->bdx", mt, wave) + beta[None, :, None])
        .reshape(B, C, H, W)
        .astype(np_dt)
    )
    spec_f = np.asarray(spectrogram, np.float32)
    spec16 = spec_f.astype(np_dt)
    # st4[b, w0, h, j, c] = spec[b, c, h, 128*j + w0]
    st4 = np.ascontiguousarray(
        spec16.reshape(B, C, H, 4, 128).transpose(0, 4, 2, 3, 1)
    )
    return t2p, spec16, st4


_NC_CACHE = {}


def _get_nc(n_h=H, rep=1, **bkw):
    key = (n_h, rep, tuple(sorted(bkw.items())))
    if key not in _NC_CACHE:
        _NC_CACHE[key] = build_module(n_h, rep, **bkw)
    return _NC_CACHE[key]


def run_device(
    waveform, spectrogram, w1, b1, w2, b2, n_h=H, rep=1, bkw=None, **run_kwargs
):
    """Shard over batch, run on 8 cores, gather. Returns (output, results)."""
    bkw = bkw or {}
    np_dt = np.float16 if bkw.get("dt", FP16) == FP16 else None
    t2p, spec16, st4 = host_prep(waveform, spectrogram, w1, b1, w2, b2, np_dt)

    in_maps = [
        {
            "t2p": np.ascontiguousarray(t2p[b, :, :n_h, :]),
            "spec": np.ascontiguousarray(spec16[b, :, :n_h, :]),
            "st4": np.ascontiguousarray(st4[b, :, :n_h, :, :]),
        }
        for b in range(B)
    ]
    nc = _get_nc(n_h, rep, **bkw)
    res = run_bass_kernel_spmd(nc, in_maps, core_ids=list(range(N_CORES)), **run_kwargs)
    output = np.stack([res.results[b]["out"] for b in range(B)], axis=0)
    return output, res


def kernel(waveform, spectrogram, w1, b1, w2, b2):
    output, _ = run_device(waveform, spectrogram, w1, b1, w2, b2)
    return output.astype(np.float32)


# revision 8
# speedup vs baseline: 26.2457x; 15.8226x over previous
"""Trainium2 Bass kernel for nn_CrossAttention2D (v3: host-T2, fp16, wide exp).

Reference computation (per batch b, row h):
    Q = w1 @ Xw + b1          (Xw = waveform[b,:,h,:]  [C=128, W=512])
    K = w2 @ Xs + b2          (Xs = spectrogram[b,:,h,:])
    S = Q^T K * 1/sqrt(F)     [512, 512]
    P = softmax(S, axis=-1)
    out[b,:,h,:] = Xs @ P^T   [C, W]

Device algorithm (S^T layout so softmax needs no transposes):
    S^T = Xs^T T2'   with  T2' = (w1^T w2)^T Xw + (w2^T b1) 1^T
precomputed on the HOST (it is a tiny [128,128] map over the waveform and
folds the only softmax-surviving bias), shipped as fp16.  Per row h:
    st[p] = Xs[:,pair p]^T T2'        (4 matmuls into 2x [128,2,512] PSUM)
    P~[p] = exp(scale*st[p] - 3)      (2 wide ACT exps, fp16 out; the -3
                                       cancels in the softmax ratio)
    rb   += ones^T P~[p]              (4 matmuls, row sums broadcast to 128)
    o    += V^T[kc] P~[p]             (4 matmuls, V^T host-pretransposed fp16)
    out   = o / rb                    (single DVE divide, fp16 out)
All device matmuls are fp16 (10-bit mantissa beats bf16 at identical PE
cost); accumulation is fp32 in PSUM.  Output is fp16, host-upcast.

The emission is software-pipelined one h ahead: body(i) issues divide(i-1)
first on the otherwise-idle DVE, then the score matmuls + exps of row i+1,
then the AV/row-sum matmuls of row i.  The PE stream is 12 N=512 matmuls
per row with no PSUM->SBUF copies on the critical path.

Sharding: data-parallel over batch B=8 across 8 NeuronCores (one batch
image per core, small weights replicated). No collectives.
"""

import contextlib

import numpy as np

import concourse.bacc as bacc
import concourse.tile as tile
from concourse import mybir
from concourse.bass_utils import run_bass_kernel_spmd

B = 8
C = 128  # channel dim (TIME_DIM == SPEC_DIM == 128)
H = 64
W = 512
N_CORES = 8
SCALE = 1.0 / 16.0  # 1/sqrt(FEATURE_DIM=256)
SHIFT = -3.0  # exp range guard; cancels in softmax
HB = 8  # h rows per input DMA block
OB = 4  # h rows per output DMA block

FP32 = mybir.dt.float32
FP16 = mybir.dt.float16
EXP = mybir.ActivationFunctionType.Exp
DIV = mybir.AluOpType.divide
MULT = mybir.AluOpType.mult
USE_DIV = False  # walrus BIR verifier rejects AluOpType.divide on DVE


def build_module(n_h=H, rep=1, dt=FP16, wide_exp=False):
    """Build the per-core Bass module processing [C, n_h, W] inputs."""
    assert n_h % HB == 0 and HB % OB == 0 and n_h >= 2 * HB
    nc = bacc.Bacc("TRN2", target_bir_lowering=False, debug=False)

    t2p = nc.dram_tensor("t2p", [C, n_h, W], dt, kind="ExternalInput").ap()
    spec = nc.dram_tensor("spec", [C, n_h, W], dt, kind="ExternalInput").ap()
    # V^T blocks: st4[w0, h, j, c] = spec[c, h, 128*j + w0]
    st4 = nc.dram_tensor("st4", [C, n_h, 4, C], dt, kind="ExternalInput").ap()
    out = nc.dram_tensor("out", [C, n_h, W], FP16, kind="ExternalOutput").ap()

    n_blk = n_h // HB

    with tile.TileContext(nc) as tc:
        with (
            tc.tile_pool(name="consts", bufs=1) as consts,
            tc.tile_pool(name="io", bufs=4) as io,
            tc.tile_pool(name="ob", bufs=2) as ob,
            tc.tile_pool(name="work", bufs=4) as work,
            tc.tile_pool(name="ps", bufs=1, space="PSUM") as ps,
        ):
            ones_sb = consts.tile([C, C], dt, tag="ones")
            nc.vector.memset(ones_sb, 1.0)
            shift_sb = consts.tile([C, 1], FP32, tag="shift")
            nc.vector.memset(shift_sb, SHIFT)

            blocks = {}  # blk index -> (t2p, spec, st4) tiles

            def load_blk(blk):
                h0 = blk * HB
                t2_t = io.tile([C, HB, W], dt, tag="t2", name="t2_t")
                nc.sync.dma_start(t2_t, t2p[:, h0 : h0 + HB, :])
                sp_t = io.tile([C, HB, W], dt, tag="sp", name="sp_t")
                nc.sync.dma_start(sp_t, spec[:, h0 : h0 + HB, :])
                s4_t = io.tile([C, HB, 4, C], dt, tag="s4", name="s4_t")
                nc.sync.dma_start(s4_t, st4[:, h0 : h0 + HB, :, :])
                blocks[blk] = (t2_t, sp_t, s4_t)

            def st_pair(h, p, expst):
                """Two score matmuls + exp(s) for chunk pair p of row h."""
                t2 = blocks[h // HB][0][:, h % HB, :]
                sp = blocks[h // HB][1][:, h % HB, :]
                nsub = 2 if wide_exp else 1
                for s in range(2 // nsub):
                    st_ps = ps.tile([C, nsub, W], FP32, tag="st", bufs=2 * (2 // nsub))
                    for j in range(nsub):
                        kc = 2 * p + s * nsub + j
                        kblk = slice(kc * 128, (kc + 1) * 128)
                        nc.tensor.matmul(
                            st_ps[:, j, :], sp[:, kblk], t2, start=True, stop=True
                        )
                    e_out = expst[:, p, :, :] if nsub == 2 else expst[:, p, s, :]
                    nc.scalar.activation(
                        e_out,
                        st_ps if nsub == 2 else st_ps[:, 0, :],
                        EXP,
                        bias=shift_sb,
                        scale=SCALE,
                    )

            # ---- prologue: fill the pipeline ----
            load_blk(0)
            load_blk(1)
            cur_exp = work.tile([C, 2, 2, W], dt, tag="p", name="expst")
            for p in range(2):
                st_pair(0, p, cur_exp)

            o_blks = {}
            finish_prev = None  # closure finalizing row i-1 (divide + dma)

            rep_ctx = tc.For_i(0, rep, 1) if rep > 1 else contextlib.nullcontext()
            with rep_ctx:
                for i in range(n_h):
                    if i % HB == 0:
                        load_blk((i // HB + 2) % n_blk)
                    if i % OB == 0:
                        o_blks[i // OB] = ob.tile(
                            [C, OB, W], FP16, tag="o", name="o_blk"
                        )
                        if i // OB - 2 in o_blks:
                            del o_blks[i // OB - 2]
                    # row i-1 epilogue first: the idle DVE runs it while the
                    # PE streams row i+1's score matmuls
                    if finish_prev is not None:
                        finish_prev()
                        finish_prev = None

                    expst_i = cur_exp
                    nxt_exp = work.tile([C, 2, 2, W], dt, tag="p", name="expst")
                    for p in range(2):
                        st_pair((i + 1) % n_h, p, nxt_exp)

                    st4_i = blocks[i // HB][2]
                    o_ps = ps.tile([C, W], FP32, tag="o", bufs=2)
                    for kc in range(4):
                        nc.tensor.matmul(
                            o_ps,
                            st4_i[:, i % HB, kc, :],
                            expst_i[:, kc // 2, kc % 2, :],
                            start=(kc == 0),
                            stop=(kc == 3),
                        )
                    rb_ps = ps.tile([C, W], FP32, tag="rb", bufs=2)
                    for kc in range(4):
                        nc.tensor.matmul(
                            rb_ps,
                            ones_sb,
                            expst_i[:, kc // 2, kc % 2, :],
                            start=(kc == 0),
                            stop=(kc == 3),
                        )
                    cur_exp = nxt_exp

                    def finish_row(i=i, rb_ps=rb_ps, o_ps=o_ps):
                        o_blk = o_blks[i // OB]
                        if USE_DIV:
                            nc.vector.tensor_tensor(
                                o_blk[:, i % OB, :], o_ps, rb_ps, op=DIV
                            )
                        else:
                            rcb_sb = work.tile([C, W], FP32, tag="rcb")
                            nc.vector.reciprocal(rcb_sb, rb_ps)
                            nc.vector.tensor_tensor(
                                o_blk[:, i % OB, :], o_ps, rcb_sb, op=MULT
                            )
                        if i % OB == OB - 1:
                            h1 = i - (OB - 1)
                            nc.gpsimd.dma_start(out[:, h1 : h1 + OB, :], o_blk)

                    if i == n_h - 1:
                        # last body finishes itself so the loop body is
                        # self-contained across the For_i back edge
                        finish_row()
                    else:
                        finish_prev = finish_row

    nc.compile()
    return nc


def host_prep(waveform, spectrogram, w1, b1, w2, b2, np_dt=np.float16):
    """Precompute host-side tensors.

    T2' = (w1^T w2)^T Xw + (w2^T b1) 1^T  folds the 1x1-conv weights and the
    softmax-surviving bias into a single fp16 tensor shipped to the device.
    """
    w1d = np.asarray(w1, np.float64)
    w2d = np.asarray(w2, np.float64)
    b1d = np.asarray(b1, np.float64)
    mt = (w1d.T @ w2d).astype(np.float32)  # [c1, c2]
    beta = (w2d.T @ b1d).astype(np.float32)  # [c2]
    wave = np.asarray(waveform, np.float32).reshape(B, C, H * W)
    import ml_dtypes

    if np_dt is None:
        np_dt = ml_dtypes.bfloat16
    t2p = (
        (np.einsum("cd,bcx->bdx", mt, wave) + beta[None, :, None])
        .reshape(B, C, H, W)
        .astype(np_dt)
    )
    spec_f = np.asarray(spectrogram, np.float32)
    spec16 = spec_f.astype(np_dt)
    # st4[b, w0, h, j, c] = spec[b, c, h, 128*j + w0]
    st4 = np.ascontiguousarray(
        spec16.reshape(B, C, H, 4, 128).transpose(0, 4, 2, 3, 1)
    )
    return t2p, spec16, st4


_NC_CACHE = {}


def _get_nc(n_h=H, rep=1, **bkw):
    key = (n_h, rep, tuple(sorted(bkw.items())))
    if key not in _NC_CACHE:
        _NC_CACHE[key] = build_module(n_h, rep, **bkw)
    return _NC_CACHE[key]


def run_device(
    waveform, spectrogram, w1, b1, w2, b2, n_h=H, rep=1, bkw=None, **run_kwargs
):
    """Shard over batch, run on 8 cores, gather. Returns (output, results)."""
    bkw = bkw or {}
    np_dt = np.float16 if bkw.get("dt", FP16) == FP16 else None
    t2p, spec16, st4 = host_prep(waveform, spectrogram, w1, b1, w2, b2, np_dt)

    in_maps = [
        {
            "t2p": np.ascontiguousarray(t2p[b, :, :n_h, :]),
            "spec": np.ascontiguousarray(spec16[b, :, :n_h, :]),
            "st4": np.ascontiguousarray(st4[b, :, :n_h, :, :]),
        }
        for b in range(B)
    ]
    nc = _get_nc(n_h, rep, **bkw)
    res = run_bass_kernel_spmd(nc, in_maps, core_ids=list(range(N_CORES)), **run_kwargs)
    output = np.stack([res.results[b]["out"] for b in range(B)], axis=0)
    return output, res


def kernel(waveform, spectrogram, w1, b1, w2, b2):
    output, _ = run_device(waveform, spectrogram, w1, b1, w2, b2)
    return output.astype(np.float32)
